# revision 37
# baseline (speedup 1.0000x reference)
"""Trainium2 Bass kernel for nn_Attention_3719441678662.

ViT-style attention block (B=16, N=577 tokens, C=768, H=12 heads, D=64)
with a CLS-row reweighting before softmax and a second output carrying the
pre-softmax CLS->patch scores.

Distribution: data-parallel over batch. 8 NeuronCores x 2 batches each;
weights replicated. Each core computes its two batches fully on-chip.

Host-side prep (cheap casts/layout only): x padded+cast to bf16 so the
xbar dma-transpose reads it directly, weights cast to bf16, the CLS
reweight factors alpha*w+(1-alpha) staged as per-partition columns, and
b_qkv's q|k part staged as per-partition columns.

Per-core dataflow (matmul operands bf16, fp32 PSUM accumulation):
  xT[C, n]  <- dma-transpose(x_bf)
  qkT[f, n] = w_qkv[:, f].T @ xT          (f in q|k halves, 12 x 128)
  v[n, f]   = xT[:, ntile].T @ w_qkv_v    (token-major, + ones column)
  scoresT[m, n] = kT.T @ qT               (per head, m-tiled, 2-buffered)
  CLS column (n=0) reweighted in PSUM; exp via ScalarE (scale=1/8)
  patch_attn from separate [1 x m] CLS-row matmuls (off critical path)
  outT[d, n] = v_aug.T @ expT             (row 64 = softmax denominator)
  out = (outAT.T @ w_proj + b) * recip(denom)  per token partition

The two batches are interleaved in program order so the PE has qkv/proj
matmul work to fill the exp-bound stretches of the attention phase.
"""

import sys

sys.path.insert(0, "/opt/trn_rl_repo")

import numpy as np

B, N, C = 16, 577, 768
H, D = 12, 64
SCALE = D ** -0.5
ALPHA = 0.1
NCORES = 8
BPC = B // NCORES  # batches per core

KT = C // 128  # k-tiles over the C contraction
NPAD = 640  # token dim padded for the xbar dma-transpose (mult of 128)
NT = (N + 127) // 128
TOK_TILES = [(i * 128, min(128, N - i * 128)) for i in range(NT)]
NCHUNKS = [(0, 512), (512, N - 512)]  # token-axis matmul chunks (<=1 PSUM bank)
CCHUNKS = [(0, 512), (512, 256)]  # 768-wide output chunks


def prep_inputs(x, attn_weight, w_qkv, b_qkv, w_proj, b_proj):
    """Host-side casts/layout. Returns global (concat over cores) arrays."""
    import ml_dtypes

    bf = ml_dtypes.bfloat16
    x = np.asarray(x, np.float32)
    aw = np.asarray(attn_weight, np.float32)
    w_qkv = np.asarray(w_qkv, np.float32)
    b_qkv = np.asarray(b_qkv, np.float32)
    w_proj = np.asarray(w_proj, np.float32)
    b_proj = np.asarray(b_proj, np.float32)
    x_bf = np.zeros((B, NPAD, C), bf)
    x_bf[:, :N, :] = x.astype(bf)
    fcol = np.ones((B, N), np.float32)
    fcol[:, 1:] = aw * ALPHA + (1.0 - ALPHA)
    fpad = np.zeros((B, NT * 128), np.float32)
    fpad[:, :N] = fcol
    factors = np.ascontiguousarray(fpad.reshape(B, NT, 128).transpose(0, 2, 1))
    bqk_cols = np.ascontiguousarray(b_qkv[: 12 * 128].reshape(12, 128).T)  # [128, 12]
    rep = lambda a: np.concatenate([np.ascontiguousarray(a)] * NCORES, axis=0)
    return {
        "x_bf": np.ascontiguousarray(x_bf),
        "factors": factors,
        "w_qkv_bf": rep(w_qkv.astype(bf)),
        "w_proj_bf": rep(w_proj.astype(bf)),
        "bqk_cols": rep(bqk_cols),
        "b_v": rep(b_qkv[2 * C :]),
        "b_proj": rep(b_proj),
    }


def _build():
    import concourse.bass as bass
    import concourse.tile as tile
    from concourse import bacc, mybir

    f32 = mybir.dt.float32
    bf16 = mybir.dt.bfloat16

    nc = bacc.Bacc(None, target_bir_lowering=False, debug=False)

    x_d = nc.dram_tensor("x_bf", [BPC, NPAD, C], bf16, kind="ExternalInput")
    fac_d = nc.dram_tensor("factors", [BPC, 128, NT], f32, kind="ExternalInput")
    wqkv_d = nc.dram_tensor("w_qkv_bf", [C, 3 * C], bf16, kind="ExternalInput")
    wproj_d = nc.dram_tensor("w_proj_bf", [C, C], bf16, kind="ExternalInput")
    bqkc_d = nc.dram_tensor("bqk_cols", [128, 12], f32, kind="ExternalInput")
    bv_d = nc.dram_tensor("b_v", [C], f32, kind="ExternalInput")
    bproj_d = nc.dram_tensor("b_proj", [C], f32, kind="ExternalInput")
    out_d = nc.dram_tensor("out", [BPC, N, C], f32, kind="ExternalOutput")
    patch_d = nc.dram_tensor("patch_attn", [BPC, H, N - 1], f32, kind="ExternalOutput")

    from contextlib import ExitStack

    with tile.TileContext(nc) as tc, ExitStack() as ctx:
        singles = ctx.enter_context(tc.tile_pool(name="singles", bufs=1))
        xT_p = ctx.enter_context(tc.tile_pool(name="xT", bufs=2 * KT))
        qkT_p = ctx.enter_context(tc.tile_pool(name="qkT", bufs=2 * 12))
        vaug_p = ctx.enter_context(tc.tile_pool(name="vaug", bufs=2 * NT))
        expT_p = ctx.enter_context(tc.tile_pool(name="expT", bufs=3))
        outAT_p = ctx.enter_context(tc.tile_pool(name="outAT", bufs=2 * KT))
        outsb_p = ctx.enter_context(tc.tile_pool(name="outsb", bufs=3))
        row_p = ctx.enter_context(tc.tile_pool(name="rows", bufs=4))
        xrow_p = ctx.enter_context(tc.tile_pool(name="xrow", bufs=3))
        bc_p = ctx.enter_context(tc.tile_pool(name="bc", bufs=4))
        mm_p = ctx.enter_context(tc.tile_pool(name="mm", bufs=4, space="PSUM"))
        sc_p = ctx.enter_context(tc.tile_pool(name="sc", bufs=2, space="PSUM"))

        # ---- constants: identity first (gates the PE x-transposes), wqkv
        # on the HWDGE queue (gates qkv), the rest on the SWDGE queue ----
        from concourse.masks import make_identity

        ident_bf = singles.tile([128, 128], bf16)
        make_identity(nc, ident_bf)
        wqkv_sb = singles.tile([128, KT, 3 * C], bf16)

        def emit_wqkv_loads():
            # column-halves: q|k M-tiles 0-8 land first so qkv can start early
            for c0, c1 in ((0, 1152), (1152, 3 * C)):
                for k in range(KT):
                    nc.sync.dma_start(
                        out=wqkv_sb[:, k, c0:c1],
                        in_=wqkv_d[k * 128 : (k + 1) * 128, c0:c1],
                    )
        factor_sb = singles.tile([128, BPC, NT], f32)
        for b in range(BPC):
            nc.gpsimd.dma_start(out=factor_sb[:, b, :], in_=fac_d[b])
        bqk_cols = singles.tile([128, 12], f32)
        nc.gpsimd.dma_start(out=bqk_cols, in_=bqkc_d[:, :])
        vb_ap = bv_d[:]
        vb_bcast = singles.tile([128, C], f32)
        nc.gpsimd.dma_start(
            out=vb_bcast,
            in_=bass.AP(
                tensor=vb_ap.tensor, offset=vb_ap.offset, ap=[[0, 128]] + list(vb_ap.ap)
            ),
        )
        pb_ap = bproj_d[:]
        pb_bcast = singles.tile([128, C], f32)
        nc.gpsimd.dma_start(
            out=pb_bcast,
            in_=bass.AP(
                tensor=pb_ap.tensor, offset=pb_ap.offset, ap=[[0, 128]] + list(pb_ap.ap)
            ),
        )
        wproj_sb = singles.tile([128, KT, C], bf16)
        for k in range(KT):
            nc.gpsimd.dma_start(
                out=wproj_sb[:, k, :], in_=wproj_d[k * 128 : (k + 1) * 128, :]
            )

        xTs, qkTs, vaugs, outATs = {}, {}, {}, {}

        def emit_x(b):
            xTs[b] = []
            for k in range(KT):
                t = xT_p.tile([128, NPAD], bf16)
                nc.sync.dma_start(
                    out=t, in_=x_d[b, :, k * 128 : (k + 1) * 128], transpose=True
                )
                xTs[b].append(t)

        def emit_x_pe(b):
            """On-chip PE transpose path: much faster ramp for batch 0."""
            xTs[b] = [xT_p.tile([128, NPAD], bf16, name="xT") for _ in range(KT)]
            for t0, tlen in TOK_TILES:
                xrow = xrow_p.tile([128, C], bf16)
                nc.sync.dma_start(out=xrow[0:tlen], in_=x_d[b, t0 : t0 + tlen, :])
                for k in range(KT):
                    pt = mm_p.tile([128, 128], bf16, tag="mm", name="xtp")
                    nc.tensor.transpose(
                        pt[:, 0:tlen],
                        xrow[0:tlen, k * 128 : (k + 1) * 128],
                        ident_bf[0:tlen, 0:tlen],
                    )
                    nc.any.tensor_copy(
                        out=xTs[b][k][:, t0 : t0 + tlen], in_=pt[:, 0:tlen]
                    )

        def emit_qkv_mtile(b, mt):
            qk_t = qkTs[b][mt] = qkT_p.tile([128, N], bf16, name="qkT")
            pss = [
                mm_p.tile([128, clen], f32, tag="mm", name="qkps")
                for _, clen in NCHUNKS
            ]
            for k in range(KT):
                for (c0, clen), ps in zip(NCHUNKS, pss):
                    nc.tensor.matmul(
                        ps,
                        wqkv_sb[:, k, mt * 128 : (mt + 1) * 128],
                        xTs[b][k][:, c0 : c0 + clen],
                        start=(k == 0),
                        stop=(k == KT - 1),
                    )
            for (c0, clen), ps in zip(NCHUNKS, pss):
                nc.any.tensor_scalar_add(
                    out=qk_t[:, c0 : c0 + clen],
                    in0=ps,
                    scalar1=bqk_cols[:, mt : mt + 1],
                )

        def emit_v_tile(b, it):
            t0, tlen = TOK_TILES[it]
            va = vaugs[b][it]
            nc.vector.memset(va[0:tlen, :, D : D + 1], 1.0)
            pss = [
                mm_p.tile([128, clen], f32, tag="mm", name="vps")
                for _, clen in CCHUNKS
            ]
            for k in range(KT):
                for (c0, clen), ps in zip(CCHUNKS, pss):
                    nc.tensor.matmul(
                        ps[0:tlen],
                        xTs[b][k][:, t0 : t0 + tlen],
                        wqkv_sb[:, k, 2 * C + c0 : 2 * C + c0 + clen],
                        start=(k == 0),
                        stop=(k == KT - 1),
                    )
            for (c0, clen), ps in zip(CCHUNKS, pss):
                h0, h1 = c0 // D, (c0 + clen) // D
                nc.vector.scalar_tensor_tensor(
                    out=va[0:tlen, h0:h1, 0:D],
                    in0=ps[0:tlen].rearrange("p (h d) -> p h d", d=D),
                    scalar=1.0,
                    in1=vb_bcast[0:tlen, c0 : c0 + clen].rearrange(
                        "p (h d) -> p h d", d=D
                    ),
                    op0=mybir.AluOpType.mult,
                    op1=mybir.AluOpType.add,
                )

        def emit_cls_row(b, j, hh):
            """patch_attn row for head 2j+hh: [1, m] = q0.T @ kT, off-chain."""
            h = 2 * j + hh
            qkT = qkTs[b]
            crow = row_p.tile([1, N], f32, tag="cls", name="crow")
            ps = sc_p.tile([1, 1024], f32, tag="sc", name="clsps")
            for c0, clen in NCHUNKS:
                nc.tensor.matmul(
                    ps[0:1, c0 : c0 + clen],
                    qkT[j][hh * 64 : hh * 64 + 64, 0:1],
                    qkT[6 + j][hh * 64 : hh * 64 + 64, c0 : c0 + clen],
                    start=True,
                    stop=True,
                )
            nc.any.tensor_scalar_mul(out=crow, in0=ps[0:1, 0:N], scalar1=SCALE)
            nc.sync.dma_start(out=patch_d[b, h, :], in_=crow[0:1, 1:N])

        def emit_attn_scores(b, j):
            qkT = qkTs[b]
            expT = expT_p.tile([128, NT, 2, N], bf16, name="expT")
            for it, (t0, tlen) in enumerate(TOK_TILES):
                for hh in range(2):
                    pp = sc_p.tile([128, 1024], f32, tag="sc", name="scps")
                    for c0, clen in NCHUNKS:
                        nc.tensor.matmul(
                            pp[0:tlen, c0 : c0 + clen],
                            qkT[6 + j][hh * 64 : hh * 64 + 64, t0 : t0 + tlen],
                            qkT[j][hh * 64 : hh * 64 + 64, c0 : c0 + clen],
                            start=True,
                            stop=True,
                        )
                    # reweight the CLS-query column (n=0) before softmax
                    nc.vector.tensor_scalar_mul(
                        out=pp[0:tlen, 0:1],
                        in0=pp[0:tlen, 0:1],
                        scalar1=factor_sb[0:tlen, b, it : it + 1],
                    )
                    nc.scalar.activation(
                        out=expT[0:tlen, it, hh, :],
                        in_=pp[0:tlen, 0:N],
                        func=mybir.ActivationFunctionType.Exp,
                        scale=SCALE,
                    )
            return expT

        def emit_attn_av(b, j, expT):
            oa = outAT_p.tile([128, N], bf16, name="oa")
            for hh in range(2):
                h = 2 * j + hh
                rr = row_p.tile([1, N], f32, tag="rr")
                bc = bc_p.tile([64, N], f32)
                # chunk-outer: chunk 1's recip/broadcast/mul epilogue overlaps
                # chunk 2's accumulation matmuls
                for c0, clen in NCHUNKS:
                    av = mm_p.tile([65, clen], f32, tag="mm", name="avps")
                    for it, (t0, tlen) in enumerate(TOK_TILES):
                        nc.tensor.matmul(
                            av,
                            vaugs[b][it][0:tlen, h, :],
                            expT[0:tlen, it, hh, c0 : c0 + clen],
                            start=(it == 0),
                            stop=(it == NT - 1),
                        )
                    nc.vector.reciprocal(
                        out=rr[:, c0 : c0 + clen], in_=av[D : D + 1, :]
                    )
                    nc.gpsimd.partition_broadcast(
                        out_ap=bc[:, c0 : c0 + clen], in_ap=rr[:, c0 : c0 + clen]
                    )
                    nc.vector.tensor_mul(
                        out=oa[hh * 64 : hh * 64 + 64, c0 : c0 + clen],
                        in0=av[0:D, :],
                        in1=bc[:, c0 : c0 + clen],
                    )
            emit_cls_row(b, j, 0)
            emit_cls_row(b, j, 1)
            outATs[b].append(oa)

        def emit_attn_pair(b, j):
            emit_attn_av(b, j, emit_attn_scores(b, j))

        def emit_proj_tile(b, it):
            t0, tlen = TOK_TILES[it]
            osb = outsb_p.tile([128, C], f32)
            pss = [
                mm_p.tile([128, clen], f32, tag="mm", name="pjps")
                for _, clen in CCHUNKS
            ]
            for k in range(KT):
                for (c0, clen), ps in zip(CCHUNKS, pss):
                    nc.tensor.matmul(
                        ps[0:tlen],
                        outATs[b][k][:, t0 : t0 + tlen],
                        wproj_sb[:, k, c0 : c0 + clen],
                        start=(k == 0),
                        stop=(k == KT - 1),
                    )
            for (c0, clen), ps in zip(CCHUNKS, pss):
                nc.vector.scalar_tensor_tensor(
                    out=osb[0:tlen, c0 : c0 + clen],
                    in0=ps[0:tlen],
                    scalar=1.0,
                    in1=pb_bcast[0:tlen, c0 : c0 + clen],
                    op0=mybir.AluOpType.mult,
                    op1=mybir.AluOpType.add,
                )
            nc.sync.dma_start(out=out_d[b, t0 : t0 + tlen, :], in_=osb[0:tlen])

        # ---------------- schedule ----------------
        for b in range(BPC):
            qkTs[b], outATs[b] = {}, []
            vaugs[b] = [
                vaug_p.tile([128, H, D + 1], bf16, name="vaug") for _ in range(NT)
            ]

        emit_x_pe(0)
        emit_wqkv_loads()
        # stream b0: scores j needs only qkT tiles j and 6+j, so emit qkv in
        # pair order and start the exp stream after the first two M-tiles
        exps0 = {}
        emit_qkv_mtile(0, 0)
        emit_qkv_mtile(0, 6)
        exps0[0] = emit_attn_scores(0, 0)
        for j in (1, 2):
            emit_qkv_mtile(0, j)
            emit_qkv_mtile(0, 6 + j)
            emit_v_tile(0, j - 1)
            exps0[j] = emit_attn_scores(0, j)
        for j in (3, 4, 5):
            emit_qkv_mtile(0, j)
            emit_qkv_mtile(0, 6 + j)
            emit_v_tile(0, j - 1)
        emit_v_tile(0, 4)
        emit_x(1)
        emit_attn_av(0, 0, exps0[0])
        exps0[3] = emit_attn_scores(0, 3)
        for j in range(4, KT):
            emit_qkv_mtile(1, 2 * (j - 4))
            emit_qkv_mtile(1, 2 * (j - 4) + 1)
            emit_attn_av(0, j - 3, exps0[j - 3])
            exps0[j] = emit_attn_scores(0, j)
        for j in (3, 4, 5):
            emit_qkv_mtile(1, 2 * (j - 1))
            emit_qkv_mtile(1, 2 * (j - 1) + 1)
            emit_attn_av(0, j, exps0[j])
        emit_qkv_mtile(1, 10)
        emit_qkv_mtile(1, 11)

        exps1 = {}
        exps1[0] = emit_attn_scores(1, 0)
        emit_v_tile(1, 0)
        emit_v_tile(1, 1)
        exps1[1] = emit_attn_scores(1, 1)
        emit_v_tile(1, 2)
        emit_v_tile(1, 3)
        exps1[2] = emit_attn_scores(1, 2)
        emit_v_tile(1, 4)
        emit_attn_av(1, 0, exps1[0])
        emit_proj_tile(0, 0)
        for j in range(3, KT):
            exps1[j] = emit_attn_scores(1, j)
            emit_attn_av(1, j - 2, exps1[j - 2])
            emit_proj_tile(0, j - 2)
        emit_attn_av(1, KT - 2, exps1[KT - 2])
        emit_proj_tile(0, NT - 1)
        emit_attn_av(1, KT - 1, exps1[KT - 1])
        for it in range(NT):
            emit_proj_tile(1, it)

    return nc


_STATE = {}


def _get_nc():
    if "nc" not in _STATE:
        nc = _build()
        if not nc.is_finalized():
            nc.finalize()
        _STATE["nc"] = nc
    return _STATE["nc"]


def _get_exec():
    """Build (once) a cached jitted shard_map executable over the 8 cores."""
    if "exec" in _STATE:
        return _STATE["exec"]

    import jax
    from jax.experimental.shard_map import shard_map
    from jax.sharding import Mesh, NamedSharding, PartitionSpec

    from concourse import bass2jax, mybir

    bass2jax.install_neuronx_cc_hook()
    nc = _get_nc()

    in_names, out_names, out_avals = [], [], []
    part_name = nc.partition_id_tensor.name if nc.partition_id_tensor else None
    for alloc in nc.m.functions[0].allocations:
        if not isinstance(alloc, mybir.MemoryLocationSet):
            continue
        name = alloc.memorylocations[0].name
        if alloc.kind == "ExternalInput":
            if name != part_name:
                in_names.append(name)
        elif alloc.kind == "ExternalOutput":
            out_names.append(name)
            out_avals.append(
                jax.core.ShapedArray(
                    tuple(alloc.tensor_shape), mybir.dt.np(alloc.dtype)
                )
            )
    n_params = len(in_names)
    all_in_names = list(in_names) + list(out_names)
    if part_name is not None:
        all_in_names.append(part_name)

    def _body(*args):
        operands = list(args)
        if part_name is not None:
            operands.append(bass2jax.partition_id_tensor())
        outs = bass2jax._bass_exec_p.bind(
            *operands,
            out_avals=tuple(out_avals),
            in_names=tuple(all_in_names),
            out_names=tuple(out_names),
            lowering_input_output_aliases=(),
            sim_require_finite=True,
            sim_require_nnan=True,
            nc=nc,
        )
        return tuple(outs)

    devices = jax.devices()[:NCORES]
    mesh = Mesh(np.asarray(devices), ("core",))
    n_outs = len(out_names)
    donate = tuple(range(n_params, n_params + n_outs))
    sharded = jax.jit(
        shard_map(
            _body,
            mesh=mesh,
            in_specs=(PartitionSpec("core"),) * (n_params + n_outs),
            out_specs=(PartitionSpec("core"),) * n_outs,
            check_rep=False,
        ),
        donate_argnums=donate,
        keep_unused=True,
    )
    _STATE["exec"] = {
        "fn": sharded,
        "in_names": in_names,
        "out_names": out_names,
        "out_avals": out_avals,
        "sharding": NamedSharding(mesh, PartitionSpec("core")),
    }
    return _STATE["exec"]


def _concat_inputs(x, attn_weight, w_qkv, b_qkv, w_proj, b_proj):
    per_name = prep_inputs(x, attn_weight, w_qkv, b_qkv, w_proj, b_proj)
    ex = _get_exec()
    return [per_name[name] for name in ex["in_names"]]


def _zero_outs():
    ex = _get_exec()
    return [
        np.zeros((NCORES * a.shape[0], *a.shape[1:]), a.dtype) for a in ex["out_avals"]
    ]


def kernel(x, attn_weight, w_qkv, b_qkv, w_proj, b_proj):
    ex = _get_exec()
    ins = _concat_inputs(x, attn_weight, w_qkv, b_qkv, w_proj, b_proj)
    outs = ex["fn"](*ins, *_zero_outs())
    res = {name: np.asarray(o) for name, o in zip(ex["out_names"], outs)}
    return res["out"], res["patch_attn"]


# revision 39
# speedup vs baseline: 1.3045x; 1.3045x over previous
"""Trainium2 Bass kernel for nn_Attention_3719441678662.

ViT-style attention block (B=16, N=577 tokens, C=768, H=12 heads, D=64)
with a CLS-row reweighting before softmax and a second output carrying the
pre-softmax CLS->patch scores.

Distribution: data-parallel over batch. 8 NeuronCores x 2 batches each;
weights replicated. Each core computes its two batches fully on-chip.

Host-side prep (cheap casts/layout only): x padded+cast to bf16 so the
xbar dma-transpose reads it directly, weights cast to bf16, the CLS
reweight factors alpha*w+(1-alpha) staged as per-partition columns, and
b_qkv's q|k part staged as per-partition columns.

Per-core dataflow (matmul operands bf16, fp32 PSUM accumulation):
  xT[C, n]  <- dma-transpose(x_bf)
  qkT[f, n] = w_qkv[:, f].T @ xT          (f in q|k halves, 12 x 128)
  v[n, f]   = xT[:, ntile].T @ w_qkv_v    (token-major, + ones column)
  scoresT[m, n] = kT.T @ qT               (per head, m-tiled, 2-buffered)
  CLS column (n=0) reweighted in PSUM; exp via ScalarE (scale=1/8)
  patch_attn from separate [1 x m] CLS-row matmuls (off critical path)
  outT[d, n] = v_aug.T @ expT             (row 64 = softmax denominator)
  out = (outAT.T @ w_proj + b) * recip(denom)  per token partition

The two batches are interleaved in program order so the PE has qkv/proj
matmul work to fill the exp-bound stretches of the attention phase.
"""

import sys

sys.path.insert(0, "/opt/trn_rl_repo")

import numpy as np

B, N, C = 16, 577, 768
H, D = 12, 64
SCALE = D ** -0.5
ALPHA = 0.1
NCORES = 8
BPC = B // NCORES  # batches per core

KT = C // 128  # k-tiles over the C contraction
NPAD = 640  # token dim padded for the xbar dma-transpose (mult of 128)
NT = (N + 127) // 128
TOK_TILES = [(i * 128, min(128, N - i * 128)) for i in range(NT)]
NCHUNKS = [(0, 512), (512, N - 512)]  # token-axis matmul chunks (<=1 PSUM bank)
CCHUNKS = [(0, 512), (512, 256)]  # 768-wide output chunks


def prep_inputs(x, attn_weight, w_qkv, b_qkv, w_proj, b_proj):
    """Host-side casts/layout. Returns global (concat over cores) arrays."""
    import ml_dtypes

    bf = ml_dtypes.bfloat16
    x = np.asarray(x, np.float32)
    aw = np.asarray(attn_weight, np.float32)
    w_qkv = np.asarray(w_qkv, np.float32)
    b_qkv = np.asarray(b_qkv, np.float32)
    w_proj = np.asarray(w_proj, np.float32)
    b_proj = np.asarray(b_proj, np.float32)
    x_bf = np.zeros((B, NPAD, C), bf)
    x_bf[:, :N, :] = x.astype(bf)
    fcol = np.ones((B, N), np.float32)
    fcol[:, 1:] = aw * ALPHA + (1.0 - ALPHA)
    fpad = np.zeros((B, NT * 128), np.float32)
    fpad[:, :N] = fcol
    factors = np.ascontiguousarray(fpad.reshape(B, NT, 128).transpose(0, 2, 1))
    bqk_cols = np.ascontiguousarray(b_qkv[: 12 * 128].reshape(12, 128).T)  # [128, 12]
    ident = np.eye(128, dtype=bf)
    rep = lambda a: np.concatenate([np.ascontiguousarray(a)] * NCORES, axis=0)
    return {
        "x_bf": np.ascontiguousarray(x_bf),
        "factors": factors,
        "w_qkv_bf": rep(w_qkv.astype(bf)),
        "w_proj_bf": rep(w_proj.astype(bf)),
        "bqk_cols": rep(bqk_cols),
        "ident": rep(ident),
        "b_v": rep(b_qkv[2 * C :]),
        "b_proj": rep(b_proj),
    }


def _build():
    import concourse.bass as bass
    import concourse.tile as tile
    from concourse import bacc, mybir

    f32 = mybir.dt.float32
    bf16 = mybir.dt.bfloat16

    nc = bacc.Bacc(None, target_bir_lowering=False, debug=False)

    x_d = nc.dram_tensor("x_bf", [BPC, NPAD, C], bf16, kind="ExternalInput")
    fac_d = nc.dram_tensor("factors", [BPC, 128, NT], f32, kind="ExternalInput")
    wqkv_d = nc.dram_tensor("w_qkv_bf", [C, 3 * C], bf16, kind="ExternalInput")
    wproj_d = nc.dram_tensor("w_proj_bf", [C, C], bf16, kind="ExternalInput")
    bqkc_d = nc.dram_tensor("bqk_cols", [128, 12], f32, kind="ExternalInput")
    ident_d = nc.dram_tensor("ident", [128, 128], bf16, kind="ExternalInput")
    bv_d = nc.dram_tensor("b_v", [C], f32, kind="ExternalInput")
    bproj_d = nc.dram_tensor("b_proj", [C], f32, kind="ExternalInput")
    out_d = nc.dram_tensor("out", [BPC, N, C], f32, kind="ExternalOutput")
    patch_d = nc.dram_tensor("patch_attn", [BPC, H, N - 1], f32, kind="ExternalOutput")

    from contextlib import ExitStack

    with tile.TileContext(nc) as tc, ExitStack() as ctx:
        singles = ctx.enter_context(tc.tile_pool(name="singles", bufs=1))
        xT_p = ctx.enter_context(tc.tile_pool(name="xT", bufs=2 * KT))
        qkT_p = ctx.enter_context(tc.tile_pool(name="qkT", bufs=2 * 12))
        vaug_p = ctx.enter_context(tc.tile_pool(name="vaug", bufs=2 * NT))
        expT_p = ctx.enter_context(tc.tile_pool(name="expT", bufs=3))
        outAT_p = ctx.enter_context(tc.tile_pool(name="outAT", bufs=2 * KT))
        outsb_p = ctx.enter_context(tc.tile_pool(name="outsb", bufs=3))
        row_p = ctx.enter_context(tc.tile_pool(name="rows", bufs=4))
        xrow_p = ctx.enter_context(tc.tile_pool(name="xrow", bufs=3))
        bc_p = ctx.enter_context(tc.tile_pool(name="bc", bufs=4))
        mm_p = ctx.enter_context(tc.tile_pool(name="mm", bufs=4, space="PSUM"))
        sc_p = ctx.enter_context(tc.tile_pool(name="sc", bufs=2, space="PSUM"))

        # ---- constants: identity first (gates the PE x-transposes), wqkv
        # on the HWDGE queue (gates qkv), the rest on the SWDGE queue ----
        ident_bf = singles.tile([128, 128], bf16)
        nc.gpsimd.dma_start(out=ident_bf, in_=ident_d[:, :])
        wqkv_sb = singles.tile([128, KT, 3 * C], bf16)

        def emit_wqkv_loads():
            # column-halves: q|k M-tiles 0-8 land first so qkv can start early
            for c0, c1 in ((0, 1152), (1152, 3 * C)):
                for k in range(KT):
                    nc.sync.dma_start(
                        out=wqkv_sb[:, k, c0:c1],
                        in_=wqkv_d[k * 128 : (k + 1) * 128, c0:c1],
                    )
        factor_sb = singles.tile([128, BPC, NT], f32)
        for b in range(BPC):
            nc.gpsimd.dma_start(out=factor_sb[:, b, :], in_=fac_d[b])
        bqk_cols = singles.tile([128, 12], f32)
        nc.gpsimd.dma_start(out=bqk_cols, in_=bqkc_d[:, :])
        vb_ap = bv_d[:]
        vb_bcast = singles.tile([128, C], f32)
        nc.gpsimd.dma_start(
            out=vb_bcast,
            in_=bass.AP(
                tensor=vb_ap.tensor, offset=vb_ap.offset, ap=[[0, 128]] + list(vb_ap.ap)
            ),
        )
        pb_ap = bproj_d[:]
        pb_bcast = singles.tile([128, C], f32)
        nc.gpsimd.dma_start(
            out=pb_bcast,
            in_=bass.AP(
                tensor=pb_ap.tensor, offset=pb_ap.offset, ap=[[0, 128]] + list(pb_ap.ap)
            ),
        )
        wproj_sb = singles.tile([128, KT, C], bf16)
        for k in range(KT):
            nc.gpsimd.dma_start(
                out=wproj_sb[:, k, :], in_=wproj_d[k * 128 : (k + 1) * 128, :]
            )

        xTs, qkTs, vaugs, outATs = {}, {}, {}, {}

        def emit_x(b):
            xTs[b] = []
            for k in range(KT):
                t = xT_p.tile([128, NPAD], bf16)
                nc.sync.dma_start(
                    out=t, in_=x_d[b, :, k * 128 : (k + 1) * 128], transpose=True
                )
                xTs[b].append(t)

        def emit_x_pe(b):
            """On-chip PE transpose path: much faster ramp for batch 0."""
            xTs[b] = [xT_p.tile([128, NPAD], bf16, name="xT") for _ in range(KT)]
            for t0, tlen in TOK_TILES:
                xrow = xrow_p.tile([128, C], bf16)
                nc.sync.dma_start(out=xrow[0:tlen], in_=x_d[b, t0 : t0 + tlen, :])
                for k in range(KT):
                    pt = mm_p.tile([128, 128], bf16, tag="mm", name="xtp")
                    nc.tensor.transpose(
                        pt[:, 0:tlen],
                        xrow[0:tlen, k * 128 : (k + 1) * 128],
                        ident_bf[0:tlen, 0:tlen],
                    )
                    nc.any.tensor_copy(
                        out=xTs[b][k][:, t0 : t0 + tlen], in_=pt[:, 0:tlen]
                    )

        def emit_qkv_mtile(b, mt):
            qk_t = qkTs[b][mt] = qkT_p.tile([128, N], bf16, name="qkT")
            pss = [
                mm_p.tile([128, clen], f32, tag="mm", name="qkps")
                for _, clen in NCHUNKS
            ]
            for k in range(KT):
                for (c0, clen), ps in zip(NCHUNKS, pss):
                    nc.tensor.matmul(
                        ps,
                        wqkv_sb[:, k, mt * 128 : (mt + 1) * 128],
                        xTs[b][k][:, c0 : c0 + clen],
                        start=(k == 0),
                        stop=(k == KT - 1),
                    )
            for (c0, clen), ps in zip(NCHUNKS, pss):
                nc.any.tensor_scalar_add(
                    out=qk_t[:, c0 : c0 + clen],
                    in0=ps,
                    scalar1=bqk_cols[:, mt : mt + 1],
                )

        def emit_v_tile(b, it):
            t0, tlen = TOK_TILES[it]
            va = vaugs[b][it]
            nc.vector.memset(va[0:tlen, :, D : D + 1], 1.0)
            pss = [
                mm_p.tile([128, clen], f32, tag="mm", name="vps")
                for _, clen in CCHUNKS
            ]
            for k in range(KT):
                for (c0, clen), ps in zip(CCHUNKS, pss):
                    nc.tensor.matmul(
                        ps[0:tlen],
                        xTs[b][k][:, t0 : t0 + tlen],
                        wqkv_sb[:, k, 2 * C + c0 : 2 * C + c0 + clen],
                        start=(k == 0),
                        stop=(k == KT - 1),
                    )
            for (c0, clen), ps in zip(CCHUNKS, pss):
                h0, h1 = c0 // D, (c0 + clen) // D
                nc.vector.scalar_tensor_tensor(
                    out=va[0:tlen, h0:h1, 0:D],
                    in0=ps[0:tlen].rearrange("p (h d) -> p h d", d=D),
                    scalar=1.0,
                    in1=vb_bcast[0:tlen, c0 : c0 + clen].rearrange(
                        "p (h d) -> p h d", d=D
                    ),
                    op0=mybir.AluOpType.mult,
                    op1=mybir.AluOpType.add,
                )

        def emit_cls_row(b, j, hh):
            """patch_attn row for head 2j+hh: [1, m] = q0.T @ kT, off-chain."""
            h = 2 * j + hh
            qkT = qkTs[b]
            crow = row_p.tile([1, N], f32, tag="cls", name="crow")
            ps = sc_p.tile([1, 1024], f32, tag="sc", name="clsps")
            for c0, clen in NCHUNKS:
                nc.tensor.matmul(
                    ps[0:1, c0 : c0 + clen],
                    qkT[j][hh * 64 : hh * 64 + 64, 0:1],
                    qkT[6 + j][hh * 64 : hh * 64 + 64, c0 : c0 + clen],
                    start=True,
                    stop=True,
                )
            nc.any.tensor_scalar_mul(out=crow, in0=ps[0:1, 0:N], scalar1=SCALE)
            nc.sync.dma_start(out=patch_d[b, h, :], in_=crow[0:1, 1:N])

        def emit_attn_scores(b, j):
            qkT = qkTs[b]
            expT = expT_p.tile([128, NT, 2, N], bf16, name="expT")
            for it, (t0, tlen) in enumerate(TOK_TILES):
                for hh in range(2):
                    pp = sc_p.tile([128, 1024], f32, tag="sc", name="scps")
                    for c0, clen in NCHUNKS:
                        nc.tensor.matmul(
                            pp[0:tlen, c0 : c0 + clen],
                            qkT[6 + j][hh * 64 : hh * 64 + 64, t0 : t0 + tlen],
                            qkT[j][hh * 64 : hh * 64 + 64, c0 : c0 + clen],
                            start=True,
                            stop=True,
                        )
                    # reweight the CLS-query column (n=0) before softmax
                    nc.vector.tensor_scalar_mul(
                        out=pp[0:tlen, 0:1],
                        in0=pp[0:tlen, 0:1],
                        scalar1=factor_sb[0:tlen, b, it : it + 1],
                    )
                    nc.scalar.activation(
                        out=expT[0:tlen, it, hh, :],
                        in_=pp[0:tlen, 0:N],
                        func=mybir.ActivationFunctionType.Exp,
                        scale=SCALE,
                    )
            return expT

        def emit_attn_av(b, j, expT):
            oa = outAT_p.tile([128, N], bf16, name="oa")
            for hh in range(2):
                h = 2 * j + hh
                rr = row_p.tile([1, N], f32, tag="rr")
                bc = bc_p.tile([64, N], f32)
                # chunk-outer: chunk 1's recip/broadcast/mul epilogue overlaps
                # chunk 2's accumulation matmuls
                for c0, clen in NCHUNKS:
                    av = mm_p.tile([65, clen], f32, tag="mm", name="avps")
                    for it, (t0, tlen) in enumerate(TOK_TILES):
                        nc.tensor.matmul(
                            av,
                            vaugs[b][it][0:tlen, h, :],
                            expT[0:tlen, it, hh, c0 : c0 + clen],
                            start=(it == 0),
                            stop=(it == NT - 1),
                        )
                    nc.vector.reciprocal(
                        out=rr[:, c0 : c0 + clen], in_=av[D : D + 1, :]
                    )
                    nc.gpsimd.partition_broadcast(
                        out_ap=bc[:, c0 : c0 + clen], in_ap=rr[:, c0 : c0 + clen]
                    )
                    nc.vector.tensor_mul(
                        out=oa[hh * 64 : hh * 64 + 64, c0 : c0 + clen],
                        in0=av[0:D, :],
                        in1=bc[:, c0 : c0 + clen],
                    )
            emit_cls_row(b, j, 0)
            emit_cls_row(b, j, 1)
            outATs[b].append(oa)

        def emit_attn_pair(b, j):
            emit_attn_av(b, j, emit_attn_scores(b, j))

        def emit_proj_tile(b, it):
            t0, tlen = TOK_TILES[it]
            osb = outsb_p.tile([128, C], f32)
            pss = [
                mm_p.tile([128, clen], f32, tag="mm", name="pjps")
                for _, clen in CCHUNKS
            ]
            for k in range(KT):
                for (c0, clen), ps in zip(CCHUNKS, pss):
                    nc.tensor.matmul(
                        ps[0:tlen],
                        outATs[b][k][:, t0 : t0 + tlen],
                        wproj_sb[:, k, c0 : c0 + clen],
                        start=(k == 0),
                        stop=(k == KT - 1),
                    )
            for (c0, clen), ps in zip(CCHUNKS, pss):
                nc.vector.scalar_tensor_tensor(
                    out=osb[0:tlen, c0 : c0 + clen],
                    in0=ps[0:tlen],
                    scalar=1.0,
                    in1=pb_bcast[0:tlen, c0 : c0 + clen],
                    op0=mybir.AluOpType.mult,
                    op1=mybir.AluOpType.add,
                )
            nc.sync.dma_start(out=out_d[b, t0 : t0 + tlen, :], in_=osb[0:tlen])

        # ---------------- schedule ----------------
        for b in range(BPC):
            qkTs[b], outATs[b] = {}, []
            vaugs[b] = [
                vaug_p.tile([128, H, D + 1], bf16, name="vaug") for _ in range(NT)
            ]

        emit_x_pe(0)
        emit_wqkv_loads()
        # stream b0: scores j needs only qkT tiles j and 6+j, so emit qkv in
        # pair order and start the exp stream after the first two M-tiles
        exps0 = {}
        emit_qkv_mtile(0, 0)
        emit_qkv_mtile(0, 6)
        exps0[0] = emit_attn_scores(0, 0)
        for j in (1, 2):
            emit_qkv_mtile(0, j)
            emit_qkv_mtile(0, 6 + j)
            emit_v_tile(0, j - 1)
            exps0[j] = emit_attn_scores(0, j)
        for j in (3, 4, 5):
            emit_qkv_mtile(0, j)
            emit_qkv_mtile(0, 6 + j)
            emit_v_tile(0, j - 1)
        emit_v_tile(0, 4)
        emit_x(1)
        emit_attn_av(0, 0, exps0[0])
        exps0[3] = emit_attn_scores(0, 3)
        for j in range(4, KT):
            emit_qkv_mtile(1, 2 * (j - 4))
            emit_qkv_mtile(1, 2 * (j - 4) + 1)
            emit_attn_av(0, j - 3, exps0[j - 3])
            exps0[j] = emit_attn_scores(0, j)
        for j in (3, 4, 5):
            emit_qkv_mtile(1, 2 * (j - 1))
            emit_qkv_mtile(1, 2 * (j - 1) + 1)
            emit_attn_av(0, j, exps0[j])
        emit_qkv_mtile(1, 10)
        emit_qkv_mtile(1, 11)

        exps1 = {}
        exps1[0] = emit_attn_scores(1, 0)
        emit_v_tile(1, 0)
        emit_v_tile(1, 1)
        exps1[1] = emit_attn_scores(1, 1)
        emit_v_tile(1, 2)
        emit_v_tile(1, 3)
        exps1[2] = emit_attn_scores(1, 2)
        emit_v_tile(1, 4)
        emit_attn_av(1, 0, exps1[0])
        emit_proj_tile(0, 0)
        for j in range(3, KT):
            exps1[j] = emit_attn_scores(1, j)
            emit_attn_av(1, j - 2, exps1[j - 2])
            emit_proj_tile(0, j - 2)
        emit_attn_av(1, KT - 2, exps1[KT - 2])
        emit_proj_tile(0, NT - 1)
        emit_attn_av(1, KT - 1, exps1[KT - 1])
        for it in range(NT):
            emit_proj_tile(1, it)

    return nc


_STATE = {}


def _get_nc():
    if "nc" not in _STATE:
        nc = _build()
        if not nc.is_finalized():
            nc.finalize()
        _STATE["nc"] = nc
    return _STATE["nc"]


def _get_exec():
    """Build (once) a cached jitted shard_map executable over the 8 cores."""
    if "exec" in _STATE:
        return _STATE["exec"]

    import jax
    from jax.experimental.shard_map import shard_map
    from jax.sharding import Mesh, NamedSharding, PartitionSpec

    from concourse import bass2jax, mybir

    bass2jax.install_neuronx_cc_hook()
    nc = _get_nc()

    in_names, out_names, out_avals = [], [], []
    part_name = nc.partition_id_tensor.name if nc.partition_id_tensor else None
    for alloc in nc.m.functions[0].allocations:
        if not isinstance(alloc, mybir.MemoryLocationSet):
            continue
        name = alloc.memorylocations[0].name
        if alloc.kind == "ExternalInput":
            if name != part_name:
                in_names.append(name)
        elif alloc.kind == "ExternalOutput":
            out_names.append(name)
            out_avals.append(
                jax.core.ShapedArray(
                    tuple(alloc.tensor_shape), mybir.dt.np(alloc.dtype)
                )
            )
    n_params = len(in_names)
    all_in_names = list(in_names) + list(out_names)
    if part_name is not None:
        all_in_names.append(part_name)

    def _body(*args):
        operands = list(args)
        if part_name is not None:
            operands.append(bass2jax.partition_id_tensor())
        outs = bass2jax._bass_exec_p.bind(
            *operands,
            out_avals=tuple(out_avals),
            in_names=tuple(all_in_names),
            out_names=tuple(out_names),
            lowering_input_output_aliases=(),
            sim_require_finite=True,
            sim_require_nnan=True,
            nc=nc,
        )
        return tuple(outs)

    devices = jax.devices()[:NCORES]
    mesh = Mesh(np.asarray(devices), ("core",))
    n_outs = len(out_names)
    donate = tuple(range(n_params, n_params + n_outs))
    sharded = jax.jit(
        shard_map(
            _body,
            mesh=mesh,
            in_specs=(PartitionSpec("core"),) * (n_params + n_outs),
            out_specs=(PartitionSpec("core"),) * n_outs,
            check_rep=False,
        ),
        donate_argnums=donate,
        keep_unused=True,
    )
    _STATE["exec"] = {
        "fn": sharded,
        "in_names": in_names,
        "out_names": out_names,
        "out_avals": out_avals,
        "sharding": NamedSharding(mesh, PartitionSpec("core")),
    }
    return _STATE["exec"]


def _concat_inputs(x, attn_weight, w_qkv, b_qkv, w_proj, b_proj):
    per_name = prep_inputs(x, attn_weight, w_qkv, b_qkv, w_proj, b_proj)
    ex = _get_exec()
    return [per_name[name] for name in ex["in_names"]]


def _zero_outs():
    ex = _get_exec()
    return [
        np.zeros((NCORES * a.shape[0], *a.shape[1:]), a.dtype) for a in ex["out_avals"]
    ]


def kernel(x, attn_weight, w_qkv, b_qkv, w_proj, b_proj):
    ex = _get_exec()
    ins = _concat_inputs(x, attn_weight, w_qkv, b_qkv, w_proj, b_proj)
    outs = ex["fn"](*ins, *_zero_outs())
    res = {name: np.asarray(o) for name, o in zip(ex["out_names"], outs)}
    return res["out"], res["patch_attn"]
